# revision 1
# baseline (speedup 1.0000x reference)
"""Classical self-attention (head-summed scores) on 8 trn2 NeuronCores.

Math (per batch b):
    Q = x Wq; K = x Wk; V = x Wv          (W_qkv split columns 3x1024)
    S = Q K^T / 8   (full-E contraction: heads+dims summed)
    P = softmax(S, axis=-1)
    out = (P V) W_out + b_out

Sharding: 8 cores = (4 batches) x (2 query-halves). Each core gets its
batch's x rotated so its 1024 query rows come first; keys are the full
2048 rows (key order is irrelevant to the result). K/V projections are
duplicated between the 2 cores of a batch; no collectives needed.

Per-core kernel layout strategy:
  - S^T layout (keys on partitions) so the softmax reduction over keys
    becomes a ones-matmul and P^T feeds the O^T matmuls directly.
  - Softmax skips the max-subtraction (scores ~ N(0,4): exp stays well
    inside fp32 range); normalization by 1/rowsum is deferred to the
    final output projection where query rows sit on partitions.
  - All big matmuls in fp32r (tf32 datapath, full rate at free dim>=256).
  - K^T and V staged through internal DRAM to stay under SBUF; Q^T stays
    SBUF-resident so the scores phase overlaps the projection phase.
"""

import sys

sys.path.insert(0, "/opt/trn_rl_repo")

import numpy as np

import concourse.bass as bass
import concourse.mybir as mybir
import concourse.tile as tile
from concourse import bacc
from concourse.masks import make_identity

B, N, E = 4, 2048, 1024
NQ = N // 2          # query rows per core
P = 128              # partitions
FT = E // P          # 8 feature tiles (contraction for projections)
ET = E // P          # 8 embed tiles
MT = N // P          # 16 key tiles
QT = NQ // P         # 8 query tiles
MB = 4               # key tiles per projection block
NBLK = MT // MB      # 4 blocks
F32 = mybir.dt.float32
F32R = mybir.dt.float32r


def build_program():
    nc = bacc.Bacc("TRN2", target_bir_lowering=False, debug=False)
    x = nc.dram_tensor("x", [N, E], F32, kind="ExternalInput").ap()
    wqkv = nc.dram_tensor("wqkv", [E, 3 * E], F32, kind="ExternalInput").ap()
    wout = nc.dram_tensor("wout", [E, E], F32, kind="ExternalInput").ap()
    bout = nc.dram_tensor("bout", [E], F32, kind="ExternalInput").ap()
    y = nc.dram_tensor("y", [NQ, E], F32, kind="ExternalOutput").ap()

    with tile.TileContext(nc) as tc:
        _body(nc, tc, x, wqkv, wout, bout, y)
    nc.compile()
    return nc


def _body(nc, tc, x, wqkv, wout, bout, y):
    with tc.tile_pool(name="dram", bufs=1, space="DRAM") as dramp:
        kT_d = dramp.tile([E, N], F32R, name="kT_d", tag="kT_d")
        v_d = dramp.tile([N, E], F32R, name="v_d", tag="v_d")

        qTp = tc.alloc_tile_pool(name="qTp", bufs=1)
        qT = [qTp.tile([P, NQ], F32R, name=f"qT{e}", tag=f"qT{e}")
              for e in range(ET)]

        _phase_project(nc, tc, x, wqkv, kT_d, v_d, qT)

        # W_out / b_out tiles; DMAs issued at phase_scores start.
        wop = tc.alloc_tile_pool(name="wo", bufs=1)
        wo = [wop.tile([P, E], F32R, name=f"wo{e}", tag=f"wo{e}")
              for e in range(ET)]
        bo_b = wop.tile([P, E], F32, name="bo_b", tag="bo_b")
        bout_bcast = bass.AP(tensor=bout.tensor, offset=0,
                             ap=[[0, P], [1, E]])
        for e in range(ET):
            nc.gpsimd.dma_start(out=wo[e], in_=wout[e * P:(e + 1) * P, :])
        nc.sync.dma_start(out=bo_b, in_=bout_bcast)

        p_tiles, recip, pres, recp = _phase_scores(nc, tc, kT_d, qT, [])
        oT, oTp = _phase_pv(nc, tc, p_tiles, v_d, pres)
        _phase_out(nc, tc, oT, recip, wo, bo_b, y)
        wop.release()
        qTp.release()
        oTp.release()
        recp.release()


def _phase_project(nc, tc, x, wqkv, kT_d, v_d, qT):
    """x -> x^T (PE transpose), then K^T (to DRAM), Q^T (SBUF), V (DRAM)."""
    with tc.tile_pool(name="wconst", bufs=1) as wcp, \
         tc.tile_pool(name="xin", bufs=3) as xp, \
         tc.tile_pool(name="xT", bufs=2) as xTp, \
         tc.tile_pool(name="ktmp", bufs=2) as ktp, \
         tc.tile_pool(name="vtmp", bufs=2) as vtp, \
         tc.tile_pool(name="tpps", bufs=2, space="PSUM") as tpp, \
         tc.tile_pool(name="pjps", bufs=4, space="PSUM") as pjp:

        ident = wcp.tile([P, P], F32, name="ident", tag="ident")
        make_identity(nc, ident)

        # Wk first: the first projection matmuls need it soonest.
        wk, wq, wv = [], [], []
        for lst, nm, c0 in ((wk, "wk", E), (wq, "wq", 0), (wv, "wv", 2 * E)):
            for f in range(FT):
                t = wcp.tile([P, E], F32R, name=f"{nm}{f}", tag=f"{nm}{f}")
                nc.gpsimd.dma_start(
                    out=t, in_=wqkv[f * P:(f + 1) * P, c0:c0 + E])
                lst.append(t)

        for blk in range(NBLK):
            xT = xTp.tile([P, FT, MB * P], F32R, name="xT", tag="xT")
            for mt in range(MB):
                m = blk * MB + mt
                xt = xp.tile([P, E], F32, name="xt", tag="xt")
                nc.sync.dma_start(out=xt, in_=x[m * P:(m + 1) * P, :])
                for f in range(FT):
                    tp = tpp.tile([P, P], F32, name="tp", tag="tp")
                    nc.tensor.transpose(tp, xt[:, f * P:(f + 1) * P], ident)
                    nc.vector.tensor_copy(xT[:, f, mt * P:(mt + 1) * P], tp)

            # K^T block (all e rows, this block's key columns)
            for e in range(ET):
                ps = pjp.tile([P, MB * P], F32, name="pjk", tag="pj")
                for f in range(FT):
                    nc.tensor.matmul(ps, wk[f][:, e * P:(e + 1) * P],
                                     xT[:, f, :],
                                     start=(f == 0), stop=(f == FT - 1))
                kt_sb = ktp.tile([P, MB * P], F32R, name="kt_sb", tag="kt_sb")
                nc.vector.tensor_copy(kt_sb, ps)
                nc.sync.dma_start(
                    out=kT_d[e * P:(e + 1) * P, blk * MB * P:(blk + 1) * MB * P],
                    in_=kt_sb)

            # Q^T block straight into resident SBUF tiles
            if blk * MB * P < NQ:
                for e in range(ET):
                    ps = pjp.tile([P, MB * P], F32, name="pjq", tag="pj")
                    for f in range(FT):
                        nc.tensor.matmul(ps, wq[f][:, e * P:(e + 1) * P],
                                         xT[:, f, :],
                                         start=(f == 0), stop=(f == FT - 1))
                    nc.vector.tensor_copy(
                        qT[e][:, blk * MB * P:(blk + 1) * MB * P], ps)

            # V block (natural layout rows) to DRAM
            for mt in range(MB):
                m = blk * MB + mt
                vt = vtp.tile([P, E], F32R, name="vt", tag="vt")
                for h in range(2):
                    ps = pjp.tile([P, E // 2], F32, name="pjv", tag="pj")
                    for f in range(FT):
                        nc.tensor.matmul(
                            ps, xT[:, f, mt * P:(mt + 1) * P],
                            wv[f][:, h * (E // 2):(h + 1) * (E // 2)],
                            start=(f == 0), stop=(f == FT - 1))
                    nc.vector.tensor_copy(
                        vt[:, h * (E // 2):(h + 1) * (E // 2)], ps)
                nc.sync.dma_start(out=v_d[m * P:(m + 1) * P, :], in_=vt)


def _phase_scores(nc, tc, kT_d, qT, wo_loads):
    """S^T = K^T.T Q^T per key tile; P^T = exp(S^T/8); rowsums via ones-matmul."""
    kT_r = kT_d.rearrange("(e p) m -> p e m", p=P)
    recp = tc.alloc_tile_pool(name="recp", bufs=1, side="right")
    pres = tc.alloc_tile_pool(name="pres", bufs=1)
    with tc.tile_pool(name="kts", bufs=3) as ktsp, \
         tc.tile_pool(name="small", bufs=1) as smp, \
         tc.tile_pool(name="sps", bufs=3, space="PSUM") as sp, \
         tc.tile_pool(name="sumps", bufs=2, space="PSUM") as sumsp:

        ones = smp.tile([P, 1], F32, name="ones", tag="ones")
        nc.vector.memset(ones, 1.0)
        sums_acc = smp.tile([P, QT], F32, name="sums_acc", tag="sums_acc")

        p_tiles = []
        for m in range(MT):
            kt = ktsp.tile([P, ET, P], F32R, name="kt", tag="kt")
            nc.sync.dma_start(out=kt, in_=kT_r[:, :, m * P:(m + 1) * P])
            s = sp.tile([P, NQ], F32, name="s", tag="s")
            for e in range(ET):
                for h in range(2):
                    nc.tensor.matmul(
                        s[:, h * (NQ // 2):(h + 1) * (NQ // 2)],
                        kt[:, e, :],
                        qT[e][:, h * (NQ // 2):(h + 1) * (NQ // 2)],
                        start=(e == 0), stop=(e == ET - 1))
            p = pres.tile([P, NQ], F32R, name=f"p{m}", tag=f"p{m}")
            nc.scalar.activation(p, s, mybir.ActivationFunctionType.Exp,
                                 scale=0.125)
            p_tiles.append(p)
            # Row-sum the PREVIOUS tile's exp: its activation ran while
            # this tile's S matmuls were on PE, so PE never waits on ACT.
            if m > 0:
                _row_sums(nc, p_tiles[m - 1], sumsp, smp, ones, sums_acc,
                          first=(m == 1))
        _row_sums(nc, p_tiles[MT - 1], sumsp, smp, ones, sums_acc,
                  first=False)

        recip = recp.tile([P, QT], F32, name="recip", tag="recip")
        nc.vector.reciprocal(recip, sums_acc)

    return p_tiles, recip, pres, recp


def _row_sums(nc, p, sumsp, smp, ones, sums_acc, first):
    sums_m = sumsp.tile([P, QT], F32, name="sums_m", tag="sums_m")
    for q in range(QT):
        nc.tensor.matmul(sums_m[:, q:q + 1],
                         p[:, q * P:(q + 1) * P].bitcast(F32), ones,
                         start=True, stop=True)
    if first:
        nc.vector.tensor_copy(sums_acc, sums_m)
    else:
        nc.vector.tensor_tensor(out=sums_acc, in0=sums_acc,
                                in1=sums_m, op=mybir.AluOpType.add)


def _phase_pv(nc, tc, p_tiles, v_d, pres):
    """O^T[e, nq] = sum_m V[m,e]^T P^T[m,nq], accumulated in PSUM.

    e-tiles are processed in 2 groups of 4 so each group's O^T rows fit
    in PSUM ([128, NQ] x 4 = 8 banks) and V streams from DRAM only once
    per group (half its columns each time).
    """
    oTp = tc.alloc_tile_pool(name="oTp", bufs=1, side="right")
    oT = [oTp.tile([P, NQ], F32R, name=f"oT{e}", tag=f"oT{e}")
          for e in range(ET)]
    EG = ET // 2
    H = NQ // 2
    with tc.tile_pool(name="vstream", bufs=4) as vsp, \
         tc.tile_pool(name="ops", bufs=1, space="PSUM") as opp:
        for g in range(2):
            o_ps = [opp.tile([P, NQ], F32, name=f"o{j}", tag=f"o{j}")
                    for j in range(EG)]
            for m in range(MT):
                vt = vsp.tile([P, EG * P], F32R, name="vs", tag="vs")
                nc.sync.dma_start(
                    out=vt,
                    in_=v_d[m * P:(m + 1) * P, g * EG * P:(g + 1) * EG * P])
                for j in range(EG):
                    for h in range(2):
                        nc.tensor.matmul(
                            o_ps[j][:, h * H:(h + 1) * H],
                            vt[:, j * P:(j + 1) * P],
                            p_tiles[m][:, h * H:(h + 1) * H],
                            start=(m == 0), stop=(m == MT - 1))
            for j in range(EG):
                nc.vector.tensor_copy(oT[g * EG + j], o_ps[j])
    pres.release()
    return oT, oTp


def _phase_out(nc, tc, oT, recip, wo, bo_b, y):
    """y rows = (O_u W_out) * recip + b_out."""
    with tc.tile_pool(name="ysb", bufs=3) as ysp, \
         tc.tile_pool(name="yps", bufs=2, space="PSUM") as ypp:

        H = E // 2
        for nqt in range(QT):
            yps = ypp.tile([P, E], F32, name="yps", tag="yps")
            for e in range(ET):
                for h in range(2):
                    nc.tensor.matmul(
                        yps[:, h * H:(h + 1) * H],
                        oT[e][:, nqt * P:(nqt + 1) * P],
                        wo[e][:, h * H:(h + 1) * H],
                        start=(e == 0), stop=(e == ET - 1))
            ysb = ysp.tile([P, E], F32, name="ysb", tag="ysb")
            nc.vector.tensor_scalar_mul(ysb, yps, recip[:, nqt:nqt + 1])
            nc.vector.tensor_tensor(out=ysb, in0=ysb, in1=bo_b,
                                    op=mybir.AluOpType.add)
            nc.sync.dma_start(out=y[nqt * P:(nqt + 1) * P, :], in_=ysb)


_NC_CACHE = None


def _get_program():
    global _NC_CACHE
    if _NC_CACHE is None:
        _NC_CACHE = build_program()
    return _NC_CACHE


def kernel(x, W_qkv, W_out, b_out):
    from concourse.bass_utils import run_bass_kernel_spmd

    x = np.asarray(x, dtype=np.float32)
    W_qkv = np.asarray(W_qkv, dtype=np.float32)
    W_out = np.asarray(W_out, dtype=np.float32)
    b_out = np.asarray(b_out, dtype=np.float32)

    nc = _get_program()
    in_maps = []
    for c in range(8):
        b, half = divmod(c, 2)
        xb = x[b]
        xrot = np.ascontiguousarray(
            np.concatenate([xb[half * NQ:], xb[:half * NQ]], axis=0))
        in_maps.append({"x": xrot, "wqkv": W_qkv, "wout": W_out,
                       "bout": b_out})
    res = run_bass_kernel_spmd(nc, in_maps, list(range(8)))
    out = np.empty((B, N, E), dtype=np.float32)
    for c in range(8):
        b, half = divmod(c, 2)
        out[b, half * NQ:(half + 1) * NQ] = res.results[c]["y"]
    return out



# revision 6
# speedup vs baseline: 1.2950x; 1.2950x over previous
"""Classical self-attention (head-summed scores) on 8 trn2 NeuronCores.

Math (per batch b):
    Q = x Wq; K = x Wk; V = x Wv          (W_qkv split columns 3x1024)
    S = Q K^T / 8   (full-E contraction: heads+dims summed)
    P = softmax(S, axis=-1)
    out = (P V) W_out + b_out

Sharding: 8 cores = (4 batches) x (2 query-halves). Each core gets its
batch's x rotated so its 1024 query rows come first; keys are the full
2048 rows (key order is irrelevant to the result). K/V projections are
duplicated between the 2 cores of a batch; no collectives needed.

v2 design:
  - Host prepacks x^T and all weight tiles into bf16 in the exact SBUF
    layouts, so the device program is a pure matmul pipeline: no
    transposes, no dtype conversions, no DRAM staging round-trips.
  - All matmul operands bf16 (1 cyc/row, same PE rate as fp32r, half
    the SBUF/DMA bytes) with fp32 PSUM accumulation; bf16 operand
    quantization (~0.2% RMS) is far inside the 2e-2 gate.
  - Everything SBUF-resident: K^T, V, Q^T, P, O^T live in SBUF between
    phases; only inputs in / y out cross HBM.
  - Softmax skips the max-subtraction (scores ~ N(0,4): exp stays well
    inside fp32 range); row sums accumulate across all key tiles in one
    PSUM region via tiny ones-matmuls; normalization by 1/rowsum is
    deferred to the final output projection.
  - Phase order on the in-order PE queue: K proj -> Q proj -> V proj ->
    scores -> PV -> out, with DMAs scheduled just ahead of first use so
    the PE pipeline starts ~5us in and never restarts (p-state stays
    hot).
"""

import sys

sys.path.insert(0, "/opt/trn_rl_repo")

import numpy as np

import concourse.bass as bass
import concourse.mybir as mybir
import concourse.tile as tile
from concourse import bacc

B, N, E = 4, 2048, 1024
NQ = N // 2          # query rows per core
P = 128              # partitions
FT = E // P          # 8 feature tiles (contraction for projections)
ET = E // P          # 8 embed tiles
MT = N // P          # 16 key tiles
QT = NQ // P         # 8 query tiles
F32 = mybir.dt.float32
BF16 = mybir.dt.bfloat16


def build_program():
    nc = bacc.Bacc("TRN2", target_bir_lowering=False, debug=False)
    # Host-prepacked bf16 operands, already in SBUF tile layouts:
    #   xT[p, f, n]  = x_rot[n, f*128+p]
    #   wkb[p, f, e] = Wk[f*128+p, e]   (same for wqb / wvb)
    #   wob[p, e, c] = W_out[e*128+p, c]
    xT_d = nc.dram_tensor("xT", [P, FT, N], BF16, kind="ExternalInput").ap()
    wkb_d = nc.dram_tensor("wkb", [P, FT, E], BF16, kind="ExternalInput").ap()
    wqb_d = nc.dram_tensor("wqb", [P, FT, E], BF16, kind="ExternalInput").ap()
    wvb_d = nc.dram_tensor("wvb", [P, FT, E], BF16, kind="ExternalInput").ap()
    wob_d = nc.dram_tensor("wob", [P, ET, E], BF16, kind="ExternalInput").ap()
    bout = nc.dram_tensor("bout", [E], F32, kind="ExternalInput").ap()
    y = nc.dram_tensor("y", [NQ, E], F32, kind="ExternalOutput").ap()

    with tile.TileContext(nc) as tc:
        _body(nc, tc, xT_d, wkb_d, wqb_d, wvb_d, wob_d, bout, y)
    nc.compile()
    return nc


def _body(nc, tc, xT_d, wkb_d, wqb_d, wvb_d, wob_d, bout, y):
    KB = 4           # key blocks of 512 for projections
    KW = N // KB     # 512 key cols per block
    HF = FT // 2     # split f-contraction in halves for early start

    # Persistent pools first (pools are LIFO stacks per side; long-lived
    # pools must sit at the bottom)
    kTp = tc.alloc_tile_pool(name="kTp", bufs=1)
    qTp = tc.alloc_tile_pool(name="qTp", bufs=1)
    vp = tc.alloc_tile_pool(name="vp", bufs=1, side="right")
    kT = kTp.tile([P, ET, N], BF16, name="kT", tag="kT")
    qT = qTp.tile([P, ET, NQ], BF16, name="qT", tag="qT")
    v = vp.tile([P, MT, E], BF16, name="v", tag="v")

    # ---- Phase A pools: projections (released before scores/PV) ----
    xTp = tc.alloc_tile_pool(name="xTp", bufs=1)
    wp = tc.alloc_tile_pool(name="wp", bufs=1)
    xT = xTp.tile([P, FT, N], BF16, name="xT", tag="xT")
    wk = wp.tile([P, FT, E], BF16, name="wk", tag="wk")
    wq = wp.tile([P, FT, E], BF16, name="wq", tag="wq")
    wv = wp.tile([P, FT, E], BF16, name="wv", tag="wv")

    # DMA order = first-use order. Halved f-chunks let the first K
    # matmuls fire after ~2 transfers.
    for h in range(2):
        nc.sync.dma_start(out=xT[:, h * HF:(h + 1) * HF, 0:KW],
                          in_=xT_d[:, h * HF:(h + 1) * HF, 0:KW])
        nc.sync.dma_start(out=wk[:, h * HF:(h + 1) * HF, :],
                          in_=wkb_d[:, h * HF:(h + 1) * HF, :])
    for blk in range(1, KB):
        nc.sync.dma_start(out=xT[:, :, blk * KW:(blk + 1) * KW],
                          in_=xT_d[:, :, blk * KW:(blk + 1) * KW])
    nc.gpsimd.dma_start(out=wq, in_=wqb_d)
    nc.gpsimd.dma_start(out=wv, in_=wvb_d)

    with tc.tile_pool(name="pjps", bufs=6, space="PSUM") as pjp:
        # K projection: kT[:, e, kcols] = sum_f wk[:, f, e*]^T xT[:, f, kcols]
        for blk in range(KB):
            for e in range(ET):
                ps = pjp.tile([P, KW], F32, name="pjk", tag="pj")
                for f in range(FT):
                    nc.tensor.matmul(ps, wk[:, f, e * P:(e + 1) * P],
                                     xT[:, f, blk * KW:(blk + 1) * KW],
                                     start=(f == 0), stop=(f == FT - 1))
                eng = nc.vector if (e % 2 == 0) else nc.scalar
                if e % 2 == 0:
                    eng.tensor_copy(kT[:, e, blk * KW:(blk + 1) * KW], ps)
                else:
                    eng.activation(kT[:, e, blk * KW:(blk + 1) * KW], ps,
                                   mybir.ActivationFunctionType.Copy)

        # Q projection (queries = first NQ rows of rotated x)
        for blk in range(2):
            for e in range(ET):
                ps = pjp.tile([P, KW], F32, name="pjq", tag="pj")
                for f in range(FT):
                    nc.tensor.matmul(ps, wq[:, f, e * P:(e + 1) * P],
                                     xT[:, f, blk * KW:(blk + 1) * KW],
                                     start=(f == 0), stop=(f == FT - 1))
                if e % 2 == 0:
                    nc.vector.tensor_copy(qT[:, e, blk * KW:(blk + 1) * KW], ps)
                else:
                    nc.scalar.activation(qT[:, e, blk * KW:(blk + 1) * KW], ps,
                                         mybir.ActivationFunctionType.Copy)

        # V projection: v[:, m, :] rows = sum_f xT[:, f, m*]^T wv[:, f, :]
        for m in range(MT):
            for hh in range(2):
                ps = pjp.tile([P, E // 2], F32, name="pjv", tag="pj")
                for f in range(FT):
                    nc.tensor.matmul(
                        ps, xT[:, f, m * P:(m + 1) * P],
                        wv[:, f, hh * (E // 2):(hh + 1) * (E // 2)],
                        start=(f == 0), stop=(f == FT - 1))
                if hh == 0:
                    nc.vector.tensor_copy(
                        v[:, m, hh * (E // 2):(hh + 1) * (E // 2)], ps)
                else:
                    nc.scalar.activation(
                        v[:, m, hh * (E // 2):(hh + 1) * (E // 2)], ps,
                        mybir.ActivationFunctionType.Copy)

    wp.release()
    xTp.release()

    # ---- Phase B pools ----
    pp = tc.alloc_tile_pool(name="pp", bufs=1)
    oTp = tc.alloc_tile_pool(name="oTp", bufs=1, side="right")
    wop = tc.alloc_tile_pool(name="wop", bufs=1, side="right")
    smp = tc.alloc_tile_pool(name="smp", bufs=1, side="right")
    p_t = pp.tile([P, MT, NQ], BF16, name="p_t", tag="p_t")
    oT = oTp.tile([P, ET, NQ], BF16, name="oT", tag="oT")
    wo = wop.tile([P, ET, E], BF16, name="wo", tag="wo")
    bo_b = wop.tile([P, E], F32, name="bo_b", tag="bo_b")
    ones = smp.tile([P, 1], BF16, name="ones", tag="ones")
    sums = smp.tile([P, QT], F32, name="sums", tag="sums")
    recip = smp.tile([P, QT], F32, name="recip", tag="recip")

    nc.gpsimd.dma_start(out=wo, in_=wob_d)
    bout_bcast = bass.AP(tensor=bout.tensor, offset=0, ap=[[0, P], [1, E]])
    nc.gpsimd.dma_start(out=bo_b, in_=bout_bcast)
    nc.vector.memset(ones, 1.0)

    # Scores: s^T[k, q] per key tile; P = exp(s/8); rowsums via
    # ones-matmuls accumulated across all tiles in one PSUM region.
    with tc.tile_pool(name="sps", bufs=2, space="PSUM") as sp, \
         tc.tile_pool(name="sumps", bufs=1, space="PSUM") as sumsp:
        sums_ps = sumsp.tile([P, QT], F32, name="sums_ps", tag="sums_ps")
        for m in range(MT):
            s = sp.tile([P, NQ], F32, name="s", tag="s")
            for e in range(ET):
                for hh in range(2):
                    nc.tensor.matmul(
                        s[:, hh * (NQ // 2):(hh + 1) * (NQ // 2)],
                        kT[:, e, m * P:(m + 1) * P],
                        qT[:, e, hh * (NQ // 2):(hh + 1) * (NQ // 2)],
                        start=(e == 0), stop=(e == ET - 1))
            nc.scalar.activation(p_t[:, m, :], s,
                                 mybir.ActivationFunctionType.Exp,
                                 scale=0.125)
            # Row-sum the PREVIOUS tile's exp so PE never waits on ACT.
            if m > 0:
                _row_sums(nc, p_t, m - 1, ones, sums_ps, first=(m == 1),
                          last=False)
        _row_sums(nc, p_t, MT - 1, ones, sums_ps, first=False, last=True)
        nc.vector.tensor_copy(sums, sums_ps)
        nc.vector.reciprocal(recip, sums)

    # PV: oT[e, q] accumulated over all m; 2 groups of 4 e-tiles to fit
    # PSUM (4 x [128, NQ] f32 = 8 banks).
    EG = ET // 2
    H = NQ // 2
    with tc.tile_pool(name="ops", bufs=1, space="PSUM") as opp:
        for g in range(2):
            o_ps = [opp.tile([P, NQ], F32, name=f"o{j}", tag=f"o{j}")
                    for j in range(EG)]
            for m in range(MT):
                for j in range(EG):
                    e = g * EG + j
                    for hh in range(2):
                        nc.tensor.matmul(
                            o_ps[j][:, hh * H:(hh + 1) * H],
                            v[:, m, e * P:(e + 1) * P],
                            p_t[:, m, hh * H:(hh + 1) * H],
                            start=(m == 0), stop=(m == MT - 1))
            for j in range(EG):
                e = g * EG + j
                if j % 2 == 0:
                    nc.vector.tensor_copy(oT[:, e, :], o_ps[j])
                else:
                    nc.scalar.activation(oT[:, e, :], o_ps[j],
                                         mybir.ActivationFunctionType.Copy)

    # Output projection: y rows = (O_u W_out) * recip + b_out
    with tc.tile_pool(name="ysb", bufs=3) as ysp, \
         tc.tile_pool(name="yps", bufs=2, space="PSUM") as ypp:
        HE = E // 2
        for nqt in range(QT):
            yps = ypp.tile([P, E], F32, name="yps", tag="yps")
            for e in range(ET):
                for hh in range(2):
                    nc.tensor.matmul(
                        yps[:, hh * HE:(hh + 1) * HE],
                        oT[:, e, nqt * P:(nqt + 1) * P],
                        wo[:, e, hh * HE:(hh + 1) * HE],
                        start=(e == 0), stop=(e == ET - 1))
            ysb = ysp.tile([P, E], F32, name="ysb", tag="ysb")
            nc.scalar.activation(ysb, yps, mybir.ActivationFunctionType.Copy,
                                 scale=recip[:, nqt:nqt + 1])
            nc.vector.tensor_tensor(out=ysb, in0=ysb, in1=bo_b,
                                    op=mybir.AluOpType.add)
            nc.sync.dma_start(out=y[nqt * P:(nqt + 1) * P, :], in_=ysb)

    smp.release()
    wop.release()
    oTp.release()
    pp.release()
    qTp.release()
    kTp.release()
    vp.release()


def _row_sums(nc, p_t, m, ones, sums_ps, first, last):
    for q in range(QT):
        nc.tensor.matmul(sums_ps[:, q:q + 1],
                         p_t[:, m, q * P:(q + 1) * P], ones,
                         start=(first and q == 0),
                         stop=(last and q == QT - 1),
                         skip_group_check=True)


_NC_CACHE = None


def _get_program():
    global _NC_CACHE
    if _NC_CACHE is None:
        _NC_CACHE = build_program()
    return _NC_CACHE


def _pack_w(w, bf16):
    # [E, C] -> [P, E//P, C] with rows f*128+p on partition p
    return np.ascontiguousarray(
        w.reshape(FT, P, -1).transpose(1, 0, 2)).astype(bf16)


def kernel(x, W_qkv, W_out, b_out):
    import ml_dtypes
    from concourse.bass_utils import run_bass_kernel_spmd

    bf16 = ml_dtypes.bfloat16
    x = np.asarray(x, dtype=np.float32)
    W_qkv = np.asarray(W_qkv, dtype=np.float32)
    W_out = np.asarray(W_out, dtype=np.float32)
    b_out = np.asarray(b_out, dtype=np.float32)

    wkb = _pack_w(W_qkv[:, E:2 * E], bf16)
    wqb = _pack_w(W_qkv[:, 0:E], bf16)
    wvb = _pack_w(W_qkv[:, 2 * E:], bf16)
    wob = _pack_w(W_out, bf16)

    nc = _get_program()
    in_maps = []
    for c in range(8):
        b, half = divmod(c, 2)
        xb = x[b]
        xrot = np.concatenate([xb[half * NQ:], xb[:half * NQ]], axis=0)
        # xT[p, f, n] = xrot[n, f*128+p]
        xT = np.ascontiguousarray(
            xrot.T.reshape(FT, P, N).transpose(1, 0, 2)).astype(bf16)
        in_maps.append({"xT": xT, "wkb": wkb, "wqb": wqb, "wvb": wvb,
                        "wob": wob, "bout": b_out})
    res = run_bass_kernel_spmd(nc, in_maps, list(range(8)))
    out = np.empty((B, N, E), dtype=np.float32)
    for c in range(8):
        b, half = divmod(c, 2)
        out[b, half * NQ:(half + 1) * NQ] = res.results[c]["y"]
    return out
